# revision 10
# baseline (speedup 1.0000x reference)
"""MiniMax Text01 Lightning Attention — 8-core Trainium2 Bass kernel.

Sharding: token-sharded (data parallel over B*S). Each core handles 1024
contiguous tokens of the flattened (B*S) axis = 4 blocks of 256 for one batch.
The chunk-scan dependency across shards is resolved with a per-(batch,head)
kv "contribution" AllGather within each batch's 4-core group plus a cheap
post-hoc correction term (o += (q*decay*bd^i) @ kv_start).

Phases (per core):
  P0  PE-transpose x shard -> XT [2048k, 1024t] f32
  P1a q/k/gate projections (fp32r, lhsT=W, rhs=XT) -> silu/sigmoid -> bf16 DRAM parks
  P1b v projection (fp32r, lhsT=XT, rhs=Wv) -> silu*mask -> bf16 DRAM park
  P2  lightning attention per head (bf16 matmuls, f32 psum), local chunks only
  P3  AllGather of local kv contributions within batch group
  P4  kv_start combine + o corrections
  P5  RMSNorm (bn_stats) + transpose o + gate multiply -> ogT
  P6  output projection (fp32r) -> y shard
"""

from contextlib import ExitStack

import numpy as np

import concourse.bacc as bacc
import concourse.bass as bass
import concourse.mybir as mybir
import concourse.tile as tile
from concourse.bass_utils import run_bass_kernel_spmd
from concourse.masks import make_identity

F32 = mybir.dt.float32
F32R = mybir.dt.float32r
BF16 = mybir.dt.bfloat16
AF = mybir.ActivationFunctionType
OP = mybir.AluOpType

B, S, H = 2, 4096, 2048
NH, HD = 16, 128
BLOCK = 256
EPS = 1e-5
N_CORES = 8
SHARD = (B * S) // N_CORES      # 1024 tokens/core
TT = SHARD // 128               # 8 token tiles of 128
NCH = SHARD // BLOCK            # 4 local chunks
KC = H // 128                   # 16 contraction chunks
GRP = N_CORES // B              # 4 cores per batch group


def _bcast_ap(src_1d: bass.AP, parts: int = 128) -> bass.AP:
    """Partition-broadcast a 1-D AP (for DMA replication)."""
    return bass.AP(tensor=src_1d.tensor, offset=src_1d.offset,
                   ap=[[0, parts]] + list(src_1d.ap))


def _build():
    nc = bacc.Bacc("TRN2", target_bir_lowering=False, debug=False,
                   num_devices=N_CORES)

    x = nc.dram_tensor("x", [SHARD, H], F32, kind="ExternalInput").ap()
    w_qkv = nc.dram_tensor("w_qkv", [H, 3 * H], F32, kind="ExternalInput").ap()
    w_gate = nc.dram_tensor("w_gate", [H, H], F32, kind="ExternalInput").ap()
    w_out = nc.dram_tensor("w_out", [H, H], F32, kind="ExternalInput").ap()
    nw = nc.dram_tensor("nw", [H], F32, kind="ExternalInput").ap()
    mask = nc.dram_tensor("mask", [SHARD], F32, kind="ExternalInput").ap()
    qdec = nc.dram_tensor("qdec", [NH, BLOCK], F32, kind="ExternalInput").ap()
    kdec = nc.dram_tensor("kdec", [NH, BLOCK], F32, kind="ExternalInput").ap()
    diagT = nc.dram_tensor("diagT", [NH, BLOCK, BLOCK], F32,
                           kind="ExternalInput").ap()
    bd = nc.dram_tensor("bd", [NH], F32, kind="ExternalInput").ap()
    wj = nc.dram_tensor("wj", [NH, GRP], F32, kind="ExternalInput").ap()
    qdecbd = nc.dram_tensor("qdecbd", [NH, SHARD], F32,
                            kind="ExternalInput").ap()
    y = nc.dram_tensor("y", [SHARD, H], F32, kind="ExternalOutput").ap()

    with tile.TileContext(nc) as tc, ExitStack() as stack:
        consts = stack.enter_context(tc.tile_pool(name="consts", bufs=1))
        ident_f = consts.tile([128, 128], F32)
        make_identity(nc, ident_f)
        ident_b = consts.tile([128, 128], BF16)
        make_identity(nc, ident_b)
        nw_sb = consts.tile([128, KC], F32)
        nc.sync.dma_start(nw_sb, nw.rearrange("(a p) -> p a", p=128))
        mask_sb = consts.tile([128, TT], F32)
        nc.sync.dma_start(mask_sb, mask.rearrange("(a p) -> p a", p=128))
        bd_sb = consts.tile([128, NH], F32)
        nc.sync.dma_start(bd_sb, _bcast_ap(bd))
        wj_sb = consts.tile([128, NH, GRP], F32)
        nc.sync.dma_start(wj_sb, _bcast_ap(wj.rearrange("h j -> (h j)")
                                           ).rearrange("p (h j) -> p h j", h=NH))
        kdec_sb = consts.tile([128, NH, 2], F32)
        nc.sync.dma_start(kdec_sb, kdec.rearrange("h (c p) -> p h c", p=128))
        eps_sb = consts.tile([128, 1], F32)
        nc.vector.memset(eps_sb, EPS)

        dram = stack.enter_context(tc.tile_pool(name="dram", bufs=1,
                                                space="DRAM"))
        qk_park = dram.tile([2, NH, 128, SHARD], BF16)   # [q/k, head, d, t]
        g_park = dram.tile([KC, 128, SHARD], BF16)       # gateT [gch, t]
        v_park = dram.tile([TT, 128, H], BF16)           # v natural [t, vch]
        cc_in = dram.tile([NH, HD, HD], F32)
        cc_out = dram.tile([GRP, NH, HD, HD], F32)

        # ---------------- P0 + P1: projections ----------------
        with (
            tc.tile_pool(name="xt", bufs=1) as xt_pool,
            tc.tile_pool(name="xin", bufs=2) as xin_pool,
            tc.tile_pool(name="wq", bufs=3) as wq_pool,
            tc.tile_pool(name="wv", bufs=2) as wv_pool,
            tc.tile_pool(name="stage", bufs=4) as stage_pool,
            tc.tile_pool(name="ps_t", bufs=2, space="PSUM") as ps_t,
            tc.tile_pool(name="ps_mm", bufs=3, space="PSUM") as ps_mm,
            tc.tile_pool(name="ps_mv", bufs=2, space="PSUM") as ps_mv,
        ):
            xt = xt_pool.tile([128, KC, SHARD], F32R)    # XT [k, t]
            for i in range(TT):
                x_in = xin_pool.tile([128, H], F32)
                nc.sync.dma_start(x_in, x[i * 128:(i + 1) * 128, :])
                for kc in range(KC):
                    pst = ps_t.tile([128, 128], F32, tag="t128")
                    nc.tensor.transpose(pst, x_in[:, kc * 128:(kc + 1) * 128],
                                        ident_f)
                    nc.scalar.copy(xt[:, kc, i * 128:(i + 1) * 128], pst)

            # P1a: q, k, gate (transposed outputs)
            for ch in range(3 * KC):
                if ch < 2 * KC:
                    src = w_qkv[:, ch * 128:(ch + 1) * 128]
                else:
                    c2 = ch - 2 * KC
                    src = w_gate[:, c2 * 128:(c2 + 1) * 128]
                w_t = wq_pool.tile([128, KC, 128], F32R, tag="wq")
                nc.sync.dma_start(w_t, src.rearrange("(kc kp) c -> kp kc c",
                                                     kp=128).bitcast(F32R))
                for th in range(SHARD // 512):
                    psum = ps_mm.tile([128, 512], F32, tag="mm512")
                    for kc in range(KC):
                        nc.tensor.matmul(
                            psum,
                            lhsT=w_t[:, kc],
                            rhs=xt[:, kc, th * 512:(th + 1) * 512],
                            start=(kc == 0), stop=(kc == KC - 1))
                    st = stage_pool.tile([128, 512], BF16, tag="stg")
                    if ch < 2 * KC:
                        nc.scalar.activation(st, psum, AF.Silu)
                        nc.sync.dma_start(
                            qk_park[ch // KC, ch % KC][:, th * 512:(th + 1) * 512],
                            st)
                    else:
                        nc.scalar.activation(st, psum, AF.Sigmoid)
                        nc.sync.dma_start(
                            g_park[ch - 2 * KC][:, th * 512:(th + 1) * 512], st)

            # P1b: v (natural output), 256-wide vch slices
            NV = 8
            VW = H // NV
            for j in range(NV):
                wv_t = wv_pool.tile([128, KC, VW], F32R, tag="wv")
                nc.sync.dma_start(
                    wv_t,
                    w_qkv[:, 2 * H + j * VW:2 * H + (j + 1) * VW]
                    .rearrange("(kc kp) c -> kp kc c", kp=128).bitcast(F32R))
                for i in range(TT):
                    psum = ps_mv.tile([128, VW], F32, tag="mm256")
                    for kc in range(KC):
                        nc.tensor.matmul(
                            psum,
                            lhsT=xt[:, kc, i * 128:(i + 1) * 128],
                            rhs=wv_t[:, kc],
                            start=(kc == 0), stop=(kc == KC - 1))
                    st = stage_pool.tile([128, VW], BF16, tag="stgv")
                    nc.scalar.activation(st, psum, AF.Silu)
                    nc.vector.tensor_scalar_mul(st, st, mask_sb[:, i:i + 1])
                    nc.sync.dma_start(
                        v_park[i][:, j * VW:(j + 1) * VW], st)

        # ---------------- P2-P6 ----------------
        with tc.tile_pool(name="ogt", bufs=1) as ogt_pool:
            ogT = ogt_pool.tile([128, KC, SHARD], F32R)
            with tc.tile_pool(name="osb", bufs=1) as o_pool:
                o_sb = o_pool.tile([128, TT, H], F32)
                with (
                    tc.tile_pool(name="att", bufs=2) as att,
                    tc.tile_pool(name="attw", bufs=3) as attw,
                    tc.tile_pool(name="kvp", bufs=2) as kvp,
                    tc.tile_pool(name="ps_at", bufs=2, space="PSUM") as ps_at,
                    tc.tile_pool(name="ps_o", bufs=2, space="PSUM") as ps_o,
                    tc.tile_pool(name="ps_kv", bufs=2, space="PSUM") as ps_kv,
                    tc.tile_pool(name="ps_sm", bufs=2, space="PSUM") as ps_sm,
                ):
                    # ---- P2: local attention ----
                    for h in range(NH):
                        qT = att.tile([128, SHARD], BF16, tag="qh")
                        nc.sync.dma_start(qT, qk_park[0, h])
                        kT = att.tile([128, SHARD], BF16, tag="kh")
                        nc.sync.dma_start(kT, qk_park[1, h])
                        v_h = att.tile([128, TT, HD], BF16, tag="vh")
                        nc.sync.dma_start(v_h,
                                          v_park[:, :, h * HD:(h + 1) * HD]
                                          .rearrange("a p c -> p a c"))
                        dgT = att.tile([128, 2, BLOCK], F32, tag="dg")
                        nc.sync.dma_start(dgT,
                                          diagT[h].rearrange("(c p) m -> p c m",
                                                             p=128))
                        qd_rep = att.tile([128, BLOCK], F32, tag="qdr")
                        nc.sync.dma_start(qd_rep, _bcast_ap(qdec[h]))

                        kv_sb = kvp.tile([128, HD], F32, tag="kv")
                        kv_bf = kvp.tile([128, HD], BF16, tag="kvb")
                        nc.vector.memset(kv_sb, 0.0)
                        nc.gpsimd.memset(kv_bf, 0.0)

                        for i in range(NCH):
                            qTd = attw.tile([128, BLOCK], BF16, tag="qtd")
                            nc.vector.tensor_tensor(
                                qTd, qT[:, i * BLOCK:(i + 1) * BLOCK],
                                qd_rep, OP.mult)
                            at_sb = attw.tile([128, 2, BLOCK], BF16,
                                              tag="atsb")
                            for p in range(2):
                                ps = ps_at.tile([128, BLOCK], F32, tag="at")
                                nc.tensor.matmul(
                                    ps,
                                    lhsT=kT[:, i * BLOCK + p * 128:
                                            i * BLOCK + (p + 1) * 128],
                                    rhs=qT[:, i * BLOCK:(i + 1) * BLOCK],
                                    start=True, stop=True)
                                nc.vector.tensor_tensor(at_sb[:, p], ps,
                                                        dgT[:, p], OP.mult)
                            kd = attw.tile([128, 2, HD], BF16, tag="kd")
                            for p in range(2):
                                pst = ps_sm.tile([128, 128], BF16, tag="sm")
                                nc.tensor.transpose(
                                    pst, kT[:, i * BLOCK + p * 128:
                                            i * BLOCK + (p + 1) * 128],
                                    ident_b)
                                nc.scalar.activation(
                                    kd[:, p], pst, AF.Copy,
                                    scale=kdec_sb[:, h, p:p + 1])
                            for mh in range(2):
                                pso = ps_o.tile([128, HD], F32, tag="o")
                                nc.tensor.matmul(
                                    pso, lhsT=qTd[:, mh * 128:(mh + 1) * 128],
                                    rhs=kv_bf, start=True, stop=False)
                                for p in range(2):
                                    nc.tensor.matmul(
                                        pso,
                                        lhsT=at_sb[:, p,
                                                   mh * 128:(mh + 1) * 128],
                                        rhs=v_h[:, 2 * i + p],
                                        start=False, stop=(p == 1))
                                nc.scalar.copy(
                                    o_sb[:, 2 * i + mh, h * HD:(h + 1) * HD],
                                    pso)
                            pskv = ps_kv.tile([128, HD], F32, tag="kvps")
                            for p in range(2):
                                nc.tensor.matmul(pskv, lhsT=kd[:, p],
                                                 rhs=v_h[:, 2 * i + p],
                                                 start=(p == 0), stop=(p == 1))
                            nc.vector.scalar_tensor_tensor(
                                kv_sb, in0=kv_sb, scalar=bd_sb[:, h:h + 1],
                                in1=pskv, op0=OP.mult, op1=OP.add)
                            if i < NCH - 1:
                                nc.scalar.copy(kv_bf, kv_sb)
                        nc.sync.dma_start(cc_in[h], kv_sb)

                    # ---- P3: AllGather within batch group ----
                    nc.gpsimd.collective_compute(
                        "AllGather", OP.bypass,
                        replica_groups=[[0, 1, 2, 3], [4, 5, 6, 7]],
                        ins=[cc_in.opt()], outs=[cc_out.opt()])

                    # ---- P4: kv_start combine + o corrections ----
                    for h in range(NH):
                        kvs = kvp.tile([128, HD], F32, tag="kvs")
                        nc.vector.memset(kvs, 0.0)
                        for j in range(GRP):
                            cj = attw.tile([128, HD], F32, tag="ccj")
                            nc.sync.dma_start(cj, cc_out[j, h])
                            nc.vector.scalar_tensor_tensor(
                                kvs, in0=cj, scalar=wj_sb[:, h, j:j + 1],
                                in1=kvs, op0=OP.mult, op1=OP.add)
                        kvs_bf = kvp.tile([128, HD], BF16, tag="kvsb")
                        nc.scalar.copy(kvs_bf, kvs)
                        qT2 = att.tile([128, SHARD], BF16, tag="qh")
                        nc.sync.dma_start(qT2, qk_park[0, h])
                        qbd_rep = att.tile([128, SHARD], F32, tag="qbd")
                        nc.sync.dma_start(qbd_rep, _bcast_ap(qdecbd[h]))
                        qTdc = att.tile([128, SHARD], BF16, tag="qtdc")
                        nc.vector.tensor_tensor(qTdc, qT2, qbd_rep, OP.mult)
                        for m in range(TT):
                            ps = ps_sm.tile([128, 128], F32, tag="sm")
                            nc.tensor.matmul(
                                ps, lhsT=qTdc[:, m * 128:(m + 1) * 128],
                                rhs=kvs_bf, start=True, stop=True)
                            osl = o_sb[:, m, h * HD:(h + 1) * HD]
                            nc.vector.tensor_tensor(osl, osl, ps, OP.add)

                # ---- P5: norm + gate -> ogT ----
                with (
                    tc.tile_pool(name="nrm", bufs=4) as nrm,
                    tc.tile_pool(name="ps5", bufs=3, space="PSUM") as ps5,
                ):
                    for i in range(TT):
                        stats = nrm.tile([128, 4, 6], F32, tag="bst")
                        for sg in range(4):
                            nc.vector.bn_stats(
                                stats[:, sg],
                                o_sb[:, i, sg * 512:(sg + 1) * 512])
                        mv = nrm.tile([128, 2], F32, tag="mv")
                        nc.vector.bn_aggr(mv, stats)
                        msq = nrm.tile([128, 1], F32, tag="msq")
                        nc.vector.tensor_tensor(msq, mv[:, 0:1], mv[:, 0:1],
                                                OP.mult)
                        nc.vector.tensor_tensor(msq, msq, mv[:, 1:2], OP.add)
                        std = nrm.tile([128, 1], F32, tag="std")
                        nc.scalar.activation(std, msq, AF.Sqrt,
                                             bias=eps_sb[:, 0:1])
                        rstd = nrm.tile([128, 1], F32, tag="rstd")
                        nc.vector.reciprocal(rstd, std)
                        nc.vector.tensor_scalar_mul(o_sb[:, i, :],
                                                    o_sb[:, i, :], rstd)
                        for kc in range(KC):
                            pst = ps5.tile([128, 128], F32, tag="tog")
                            nc.tensor.transpose(
                                pst, o_sb[:, i, kc * 128:(kc + 1) * 128],
                                ident_f)
                            g_sb = nrm.tile([128, 128], BF16, tag="gsb")
                            nc.sync.dma_start(
                                g_sb, g_park[kc][:, i * 128:(i + 1) * 128])
                            nc.vector.scalar_tensor_tensor(
                                ogT[:, kc, i * 128:(i + 1) * 128],
                                in0=pst, scalar=nw_sb[:, kc:kc + 1], in1=g_sb,
                                op0=OP.mult, op1=OP.mult)

            # ---- P6: out projection ----
            with (
                tc.tile_pool(name="wo", bufs=2) as wo_pool,
                tc.tile_pool(name="ost", bufs=4) as ost_pool,
                tc.tile_pool(name="ps_mo", bufs=4, space="PSUM") as ps_mo,
            ):
                for j in range(4):
                    wo_t = wo_pool.tile([128, KC, 512], F32R, tag="wo")
                    nc.sync.dma_start(
                        wo_t, w_out[:, j * 512:(j + 1) * 512]
                        .rearrange("(kc kp) c -> kp kc c", kp=128).bitcast(F32R))
                    for i in range(TT):
                        psum = ps_mo.tile([128, 512], F32, tag="mo")
                        for kc in range(KC):
                            nc.tensor.matmul(
                                psum,
                                lhsT=ogT[:, kc, i * 128:(i + 1) * 128]
                                .bitcast(F32R),
                                rhs=wo_t[:, kc],
                                start=(kc == 0), stop=(kc == KC - 1))
                        ost = ost_pool.tile([128, 512], F32, tag="ost")
                        nc.scalar.copy(ost, psum)
                        nc.sync.dma_start(
                            y[i * 128:(i + 1) * 128, j * 512:(j + 1) * 512],
                            ost)

    nc.compile()
    return nc


_CACHED = None


def _get_nc():
    global _CACHED
    if _CACHED is None:
        _CACHED = _build()
    return _CACHED


def _host_tables(slope: np.ndarray):
    slope = slope.astype(np.float32)
    ar = np.arange(BLOCK, dtype=np.float32) + 1.0
    qdec = np.exp(-slope[:, None] * ar[None, :]).astype(np.float32)
    kdec = np.exp(-slope[:, None] * (BLOCK - ar)[None, :]).astype(np.float32)
    idx = ar[:, None] - ar[None, :]
    m2 = (idx >= 0).astype(np.float32)
    diag = np.exp(-slope[:, None, None] * (idx * m2)[None]) * m2[None]
    diagT = np.ascontiguousarray(diag.transpose(0, 2, 1)).astype(np.float32)
    bd = np.exp(-slope * BLOCK).astype(np.float32)
    # correction q scale: qdecbd[h, i*BLOCK + m] = qdec[h, m] * bd[h]^i
    qdecbd = np.zeros((NH, SHARD), np.float32)
    for i in range(NCH):
        qdecbd[:, i * BLOCK:(i + 1) * BLOCK] = qdec * (bd[:, None] ** i)
    return qdec, kdec, diagT, bd, qdecbd


def _make_in_maps(hidden_states, attention_mask, slope_rate, w_qkv, w_gate,
                  w_out, norm_weight):
    hs = np.ascontiguousarray(np.asarray(hidden_states, np.float32)
                              .reshape(B * S, H))
    mask = np.ascontiguousarray(np.asarray(attention_mask, np.float32)
                                .reshape(B * S))
    w_qkv = np.ascontiguousarray(np.asarray(w_qkv, np.float32))
    w_gate = np.ascontiguousarray(np.asarray(w_gate, np.float32))
    w_out = np.ascontiguousarray(np.asarray(w_out, np.float32))
    nw = np.ascontiguousarray(np.asarray(norm_weight, np.float32))
    slope = np.asarray(slope_rate, np.float32)
    qdec, kdec, diagT, bd, qdecbd = _host_tables(slope)

    in_maps = []
    for c in range(N_CORES):
        r = c % GRP
        wj = np.zeros((NH, GRP), np.float32)
        for j in range(r):
            wj[:, j] = bd ** (4 * (r - 1 - j))
        in_maps.append({
            "x": np.ascontiguousarray(hs[c * SHARD:(c + 1) * SHARD]),
            "mask": np.ascontiguousarray(mask[c * SHARD:(c + 1) * SHARD]),
            "w_qkv": w_qkv, "w_gate": w_gate, "w_out": w_out, "nw": nw,
            "qdec": qdec, "kdec": kdec, "diagT": diagT, "bd": bd,
            "wj": wj, "qdecbd": qdecbd,
        })
    return in_maps


def kernel(hidden_states, attention_mask, slope_rate, w_qkv, w_gate, w_out,
           norm_weight):
    nc = _get_nc()
    in_maps = _make_in_maps(hidden_states, attention_mask, slope_rate, w_qkv,
                            w_gate, w_out, norm_weight)

    import os
    trace = bool(int(os.environ.get("KERNEL_TRACE", "0")))
    res = run_bass_kernel_spmd(nc, in_maps, core_ids=list(range(N_CORES)),
                               trace=trace)
    kernel.last_results = res
    out = np.concatenate([res.results[c]["y"] for c in range(N_CORES)], axis=0)
    return out.reshape(B, S, H)


# revision 14
# speedup vs baseline: 1.1023x; 1.1023x over previous
"""MiniMax Text01 Lightning Attention — 8-core Trainium2 Bass kernel.

Sharding: token-sharded (data parallel over B*S). Each core handles 1024
contiguous tokens of the flattened (B*S) axis = 4 blocks of 256 for one batch.
The chunk-scan dependency across shards is resolved with a per-(batch,head)
kv "contribution" AllGather within each batch's 4-core group plus a cheap
post-hoc correction term (o += (q*decay*bd^i) @ kv_start).

Phases (per core):
  P0  PE-transpose x shard -> XT [2048k, 1024t] f32
  P1a q/k/gate projections (fp32r, lhsT=W, rhs=XT) -> silu/sigmoid -> bf16 DRAM parks
  P1b v projection (fp32r, lhsT=XT, rhs=Wv) -> silu*mask -> bf16 DRAM park
  P2  lightning attention per head (bf16 matmuls, f32 psum), local chunks only
  P3  AllGather of local kv contributions within batch group
  P4  kv_start combine + o corrections
  P5  RMSNorm (bn_stats) + transpose o + gate multiply -> ogT
  P6  output projection (fp32r) -> y shard
"""

from contextlib import ExitStack

import ml_dtypes
import numpy as np

import concourse.bacc as bacc
import concourse.bass as bass
import concourse.mybir as mybir
import concourse.tile as tile
from concourse.bass_utils import run_bass_kernel_spmd
from concourse.masks import make_identity

F32 = mybir.dt.float32
F32R = mybir.dt.float32r
BF16 = mybir.dt.bfloat16
AF = mybir.ActivationFunctionType
OP = mybir.AluOpType

B, S, H = 2, 4096, 2048
NH, HD = 16, 128
BLOCK = 256
EPS = 1e-5
N_CORES = 8
SHARD = (B * S) // N_CORES      # 1024 tokens/core
TT = SHARD // 128               # 8 token tiles of 128
NCH = SHARD // BLOCK            # 4 local chunks
KC = H // 128                   # 16 contraction chunks
GRP = N_CORES // B              # 4 cores per batch group


def _bcast_ap(src_1d: bass.AP, parts: int = 128) -> bass.AP:
    """Partition-broadcast a 1-D AP (for DMA replication)."""
    return bass.AP(tensor=src_1d.tensor, offset=src_1d.offset,
                   ap=[[0, parts]] + list(src_1d.ap))


def _build():
    nc = bacc.Bacc("TRN2", target_bir_lowering=False, debug=False,
                   num_devices=N_CORES)

    x = nc.dram_tensor("x", [SHARD, H], BF16, kind="ExternalInput").ap()
    w_qkv = nc.dram_tensor("w_qkv", [H, 3 * H], BF16,
                           kind="ExternalInput").ap()
    w_gate = nc.dram_tensor("w_gate", [H, H], BF16,
                            kind="ExternalInput").ap()
    w_out = nc.dram_tensor("w_out", [H, H], BF16,
                           kind="ExternalInput").ap()
    nw = nc.dram_tensor("nw", [H], F32, kind="ExternalInput").ap()
    mask = nc.dram_tensor("mask", [SHARD], F32, kind="ExternalInput").ap()
    qdec = nc.dram_tensor("qdec", [NH, BLOCK], F32, kind="ExternalInput").ap()
    kdec = nc.dram_tensor("kdec", [NH, BLOCK], F32, kind="ExternalInput").ap()
    diagT = nc.dram_tensor("diagT", [NH, BLOCK, BLOCK], F32,
                           kind="ExternalInput").ap()
    bd = nc.dram_tensor("bd", [NH], F32, kind="ExternalInput").ap()
    wj = nc.dram_tensor("wj", [NH, GRP], F32, kind="ExternalInput").ap()
    qdecbd = nc.dram_tensor("qdecbd", [NH, SHARD], F32,
                            kind="ExternalInput").ap()
    y = nc.dram_tensor("y", [SHARD, H], F32, kind="ExternalOutput").ap()

    with tile.TileContext(nc) as tc, ExitStack() as stack:
        consts = stack.enter_context(tc.tile_pool(name="consts", bufs=1))
        ident_f = consts.tile([128, 128], F32)
        make_identity(nc, ident_f)
        ident_b = consts.tile([128, 128], BF16)
        make_identity(nc, ident_b)
        nw_sb = consts.tile([128, KC], F32)
        nc.sync.dma_start(nw_sb, nw.rearrange("(a p) -> p a", p=128))
        mask_sb = consts.tile([128, TT], F32)
        nc.sync.dma_start(mask_sb, mask.rearrange("(a p) -> p a", p=128))
        bd_sb = consts.tile([128, NH], F32)
        nc.sync.dma_start(bd_sb, _bcast_ap(bd))
        wj_sb = consts.tile([128, NH, GRP], F32)
        nc.sync.dma_start(wj_sb, _bcast_ap(wj.rearrange("h j -> (h j)")
                                           ).rearrange("p (h j) -> p h j", h=NH))
        kdec_sb = consts.tile([128, NH, 2], F32)
        nc.sync.dma_start(kdec_sb, kdec.rearrange("h (c p) -> p h c", p=128))
        eps_sb = consts.tile([128, 1], F32)
        nc.vector.memset(eps_sb, EPS)

        dram = stack.enter_context(tc.tile_pool(name="dram", bufs=1,
                                                space="DRAM"))
        qk_park = dram.tile([2, NH, 128, SHARD], BF16)   # [q/k, head, d, t]
        g_park = dram.tile([KC, 128, SHARD], BF16)       # gateT [gch, t]
        v_park = dram.tile([TT, 128, H], BF16)           # v natural [t, vch]
        cc_in = dram.tile([NH, HD, HD], F32)
        cc_out = dram.tile([GRP, NH // 2, HD, HD], F32)
        cc_out2 = dram.tile([GRP, NH // 2, HD, HD], F32)

        # ---------------- P0 + P1: projections ----------------
        with (
            tc.tile_pool(name="xt", bufs=1) as xt_pool,
            tc.tile_pool(name="xin", bufs=2) as xin_pool,
            tc.tile_pool(name="wq", bufs=3) as wq_pool,
            tc.tile_pool(name="wv", bufs=2) as wv_pool,
            tc.tile_pool(name="stage", bufs=4) as stage_pool,
            tc.tile_pool(name="ps_t", bufs=2, space="PSUM") as ps_t,
            tc.tile_pool(name="ps_mm", bufs=3, space="PSUM") as ps_mm,
            tc.tile_pool(name="ps_mv", bufs=2, space="PSUM") as ps_mv,
        ):
            xt = xt_pool.tile([128, KC, SHARD], BF16)    # XT [k, t]
            for i in range(TT):
                x_in = xin_pool.tile([128, H], BF16)
                nc.sync.dma_start(x_in, x[i * 128:(i + 1) * 128, :])
                for kc in range(KC):
                    pst = ps_t.tile([128, 128], BF16, tag="t128")
                    nc.tensor.transpose(pst, x_in[:, kc * 128:(kc + 1) * 128],
                                        ident_b)
                    nc.scalar.copy(xt[:, kc, i * 128:(i + 1) * 128], pst)

            # P1a: q, k, gate (transposed outputs)
            for ch in range(3 * KC):
                if ch < 2 * KC:
                    src = w_qkv[:, ch * 128:(ch + 1) * 128]
                else:
                    c2 = ch - 2 * KC
                    src = w_gate[:, c2 * 128:(c2 + 1) * 128]
                w_t = wq_pool.tile([128, KC, 128], BF16, tag="wq")
                nc.sync.dma_start(w_t, src.rearrange("(kc kp) c -> kp kc c",
                                                     kp=128))
                for th in range(SHARD // 512):
                    psum = ps_mm.tile([128, 512], F32, tag="mm512")
                    for kc in range(KC):
                        nc.tensor.matmul(
                            psum,
                            lhsT=w_t[:, kc],
                            rhs=xt[:, kc, th * 512:(th + 1) * 512],
                            start=(kc == 0), stop=(kc == KC - 1))
                    st = stage_pool.tile([128, 512], BF16, tag="stg")
                    if ch < 2 * KC:
                        nc.scalar.activation(st, psum, AF.Silu)
                        nc.sync.dma_start(
                            qk_park[ch // KC, ch % KC][:, th * 512:(th + 1) * 512],
                            st)
                    else:
                        nc.scalar.activation(st, psum, AF.Sigmoid)
                        nc.sync.dma_start(
                            g_park[ch - 2 * KC][:, th * 512:(th + 1) * 512], st)

            # P1b: v (natural output), 256-wide vch slices
            NV = 4
            VW = H // NV
            for j in range(NV):
                wv_t = wv_pool.tile([128, KC, VW], BF16, tag="wv")
                nc.sync.dma_start(
                    wv_t,
                    w_qkv[:, 2 * H + j * VW:2 * H + (j + 1) * VW]
                    .rearrange("(kc kp) c -> kp kc c", kp=128))
                for i in range(TT):
                    psum = ps_mv.tile([128, VW], F32, tag="mm256")
                    for kc in range(KC):
                        nc.tensor.matmul(
                            psum,
                            lhsT=xt[:, kc, i * 128:(i + 1) * 128],
                            rhs=wv_t[:, kc],
                            start=(kc == 0), stop=(kc == KC - 1))
                    st = stage_pool.tile([128, VW], BF16, tag="stgv")
                    nc.scalar.activation(st, psum, AF.Silu)
                    nc.vector.tensor_scalar_mul(st, st, mask_sb[:, i:i + 1])
                    nc.sync.dma_start(
                        v_park[i][:, j * VW:(j + 1) * VW], st)

        # ---------------- P2-P6 ----------------
        with tc.tile_pool(name="ogt", bufs=1) as ogt_pool:
            ogT = ogt_pool.tile([128, KC, SHARD], BF16)
            with tc.tile_pool(name="osb", bufs=1) as o_pool:
                o_sb = o_pool.tile([128, TT, H], F32)
                with (
                    tc.tile_pool(name="att", bufs=2) as att,
                    tc.tile_pool(name="attw", bufs=3) as attw,
                    tc.tile_pool(name="kvp", bufs=2) as kvp,
                    tc.tile_pool(name="ps_at", bufs=2, space="PSUM") as ps_at,
                    tc.tile_pool(name="ps_o", bufs=2, space="PSUM") as ps_o,
                    tc.tile_pool(name="ps_kv", bufs=2, space="PSUM") as ps_kv,
                    tc.tile_pool(name="ps_sm", bufs=2, space="PSUM") as ps_sm,
                ):
                    # ---- P2: local attention ----
                    for h in range(NH):
                        qT = att.tile([128, SHARD], BF16, tag="qh")
                        nc.sync.dma_start(qT, qk_park[0, h])
                        kT = att.tile([128, SHARD], BF16, tag="kh")
                        nc.sync.dma_start(kT, qk_park[1, h])
                        v_h = att.tile([128, TT, HD], BF16, tag="vh")
                        nc.sync.dma_start(v_h,
                                          v_park[:, :, h * HD:(h + 1) * HD]
                                          .rearrange("a p c -> p a c"))
                        dgT = att.tile([128, 2, BLOCK], F32, tag="dg")
                        nc.sync.dma_start(dgT,
                                          diagT[h].rearrange("(c p) m -> p c m",
                                                             p=128))
                        qd_rep = att.tile([128, BLOCK], F32, tag="qdr")
                        nc.sync.dma_start(qd_rep, _bcast_ap(qdec[h]))

                        kv_sb = kvp.tile([128, HD], F32, tag="kv")
                        kv_bf = kvp.tile([128, HD], BF16, tag="kvb")

                        for i in range(NCH):
                            if i > 0:
                                qTd = attw.tile([128, BLOCK], BF16, tag="qtd")
                                nc.vector.tensor_tensor(
                                    qTd, qT[:, i * BLOCK:(i + 1) * BLOCK],
                                    qd_rep, OP.mult)
                            at_sb = attw.tile([128, 2, BLOCK], BF16,
                                              tag="atsb")
                            for p in range(2):
                                ps = ps_at.tile([128, BLOCK], F32, tag="at")
                                nc.tensor.matmul(
                                    ps,
                                    lhsT=kT[:, i * BLOCK + p * 128:
                                            i * BLOCK + (p + 1) * 128],
                                    rhs=qT[:, i * BLOCK:(i + 1) * BLOCK],
                                    start=True, stop=True)
                                nc.vector.tensor_tensor(at_sb[:, p], ps,
                                                        dgT[:, p], OP.mult)
                            kd = attw.tile([128, 2, HD], BF16, tag="kd")
                            for p in range(2):
                                pst = ps_sm.tile([128, 128], BF16, tag="sm")
                                nc.tensor.transpose(
                                    pst, kT[:, i * BLOCK + p * 128:
                                            i * BLOCK + (p + 1) * 128],
                                    ident_b)
                                nc.scalar.activation(
                                    kd[:, p], pst, AF.Copy,
                                    scale=kdec_sb[:, h, p:p + 1])
                            for mh in range(2):
                                pso = ps_o.tile([128, HD], F32, tag="o")
                                if i > 0:
                                    nc.tensor.matmul(
                                        pso,
                                        lhsT=qTd[:, mh * 128:(mh + 1) * 128],
                                        rhs=kv_bf, start=True, stop=False)
                                for p in range(2):
                                    nc.tensor.matmul(
                                        pso,
                                        lhsT=at_sb[:, p,
                                                   mh * 128:(mh + 1) * 128],
                                        rhs=v_h[:, 2 * i + p],
                                        start=(i == 0 and p == 0),
                                        stop=(p == 1))
                                nc.scalar.copy(
                                    o_sb[:, 2 * i + mh, h * HD:(h + 1) * HD],
                                    pso)
                            pskv = ps_kv.tile([128, HD], F32, tag="kvps")
                            for p in range(2):
                                nc.tensor.matmul(pskv, lhsT=kd[:, p],
                                                 rhs=v_h[:, 2 * i + p],
                                                 start=(p == 0), stop=(p == 1))
                            if i == 0:
                                nc.vector.tensor_copy(kv_sb, pskv)
                            else:
                                nc.vector.scalar_tensor_tensor(
                                    kv_sb, in0=kv_sb, scalar=bd_sb[:, h:h + 1],
                                    in1=pskv, op0=OP.mult, op1=OP.add)
                            if i < NCH - 1:
                                nc.scalar.copy(kv_bf, kv_sb)
                        nc.sync.dma_start(cc_in[h], kv_sb)
                        if h == NH // 2 - 1:
                            nc.gpsimd.collective_compute(
                                "AllGather", OP.bypass,
                                replica_groups=[[0, 1, 2, 3], [4, 5, 6, 7]],
                                ins=[cc_in[0:NH // 2].opt()],
                                outs=[cc_out.opt()])
                        elif h == NH - 1:
                            nc.gpsimd.collective_compute(
                                "AllGather", OP.bypass,
                                replica_groups=[[0, 1, 2, 3], [4, 5, 6, 7]],
                                ins=[cc_in[NH // 2:].opt()],
                                outs=[cc_out2.opt()])

                    # ---- P4: kv_start combine + o corrections ----
                    for h in range(NH):
                        kvs = kvp.tile([128, HD], F32, tag="kvs")
                        nc.vector.memset(kvs, 0.0)
                        for j in range(GRP):
                            cj = attw.tile([128, HD], F32, tag="ccj")
                            if h < NH // 2:
                                nc.sync.dma_start(cj, cc_out[j, h])
                            else:
                                nc.sync.dma_start(cj, cc_out2[j, h - NH // 2])
                            nc.vector.scalar_tensor_tensor(
                                kvs, in0=cj, scalar=wj_sb[:, h, j:j + 1],
                                in1=kvs, op0=OP.mult, op1=OP.add)
                        kvs_bf = kvp.tile([128, HD], BF16, tag="kvsb")
                        nc.scalar.copy(kvs_bf, kvs)
                        qT2 = att.tile([128, SHARD], BF16, tag="qh")
                        nc.sync.dma_start(qT2, qk_park[0, h])
                        qbd_rep = att.tile([128, SHARD], F32, tag="qbd")
                        nc.sync.dma_start(qbd_rep, _bcast_ap(qdecbd[h]))
                        qTdc = att.tile([128, SHARD], BF16, tag="qtdc")
                        nc.vector.tensor_tensor(qTdc, qT2, qbd_rep, OP.mult)
                        for m in range(TT):
                            ps = ps_sm.tile([128, 128], F32, tag="sm")
                            nc.tensor.matmul(
                                ps, lhsT=qTdc[:, m * 128:(m + 1) * 128],
                                rhs=kvs_bf, start=True, stop=True)
                            osl = o_sb[:, m, h * HD:(h + 1) * HD]
                            nc.vector.tensor_tensor(osl, osl, ps, OP.add)

                # ---- P5: norm + gate -> ogT (w_out prefetch overlaps) ----
                with (
                    tc.tile_pool(name="nrm", bufs=4) as nrm,
                    tc.tile_pool(name="ps5", bufs=3, space="PSUM") as ps5,
                    tc.tile_pool(name="wo", bufs=2) as wo_pool,
                    tc.tile_pool(name="ost", bufs=4) as ost_pool,
                    tc.tile_pool(name="ps_mo", bufs=4, space="PSUM") as ps_mo,
                ):
                    for i in range(TT):
                        stats = nrm.tile([128, 4, 6], F32, tag="bst")
                        for sg in range(4):
                            nc.vector.bn_stats(
                                stats[:, sg],
                                o_sb[:, i, sg * 512:(sg + 1) * 512])
                        mv = nrm.tile([128, 2], F32, tag="mv")
                        nc.vector.bn_aggr(mv, stats)
                        msq = nrm.tile([128, 1], F32, tag="msq")
                        nc.vector.tensor_tensor(msq, mv[:, 0:1], mv[:, 0:1],
                                                OP.mult)
                        nc.vector.tensor_tensor(msq, msq, mv[:, 1:2], OP.add)
                        std = nrm.tile([128, 1], F32, tag="std")
                        nc.scalar.activation(std, msq, AF.Sqrt,
                                             bias=eps_sb[:, 0:1])
                        rstd = nrm.tile([128, 1], F32, tag="rstd")
                        nc.vector.reciprocal(rstd, std)
                        nc.vector.tensor_scalar_mul(o_sb[:, i, :],
                                                    o_sb[:, i, :], rstd)
                        for kc in range(KC):
                            pst = ps5.tile([128, 128], F32, tag="tog")
                            nc.tensor.transpose(
                                pst, o_sb[:, i, kc * 128:(kc + 1) * 128],
                                ident_f)
                            g_sb = nrm.tile([128, 128], BF16, tag="gsb")
                            nc.sync.dma_start(
                                g_sb, g_park[kc][:, i * 128:(i + 1) * 128])
                            nc.vector.scalar_tensor_tensor(
                                ogT[:, kc, i * 128:(i + 1) * 128],
                                in0=pst, scalar=nw_sb[:, kc:kc + 1], in1=g_sb,
                                op0=OP.mult, op1=OP.mult)

                    # ---- P6: out projection ----
                    for j in range(4):
                        wo_t = wo_pool.tile([128, KC, 512], BF16, tag="wo")
                        nc.sync.dma_start(
                            wo_t, w_out[:, j * 512:(j + 1) * 512]
                            .rearrange("(kc kp) c -> kp kc c", kp=128))
                        for i in range(TT):
                            psum = ps_mo.tile([128, 512], F32, tag="mo")
                            for kc in range(KC):
                                nc.tensor.matmul(
                                    psum,
                                    lhsT=ogT[:, kc, i * 128:(i + 1) * 128],
                                    rhs=wo_t[:, kc],
                                    start=(kc == 0), stop=(kc == KC - 1))
                            ost = ost_pool.tile([128, 512], F32, tag="ost")
                            nc.scalar.copy(ost, psum)
                            nc.sync.dma_start(
                                y[i * 128:(i + 1) * 128,
                                  j * 512:(j + 1) * 512], ost)

    nc.compile()
    return nc


_CACHED = None


def _get_nc():
    global _CACHED
    if _CACHED is None:
        _CACHED = _build()
    return _CACHED


def _host_tables(slope: np.ndarray):
    slope = slope.astype(np.float32)
    ar = np.arange(BLOCK, dtype=np.float32) + 1.0
    qdec = np.exp(-slope[:, None] * ar[None, :]).astype(np.float32)
    kdec = np.exp(-slope[:, None] * (BLOCK - ar)[None, :]).astype(np.float32)
    idx = ar[:, None] - ar[None, :]
    m2 = (idx >= 0).astype(np.float32)
    diag = np.exp(-slope[:, None, None] * (idx * m2)[None]) * m2[None]
    diagT = np.ascontiguousarray(diag.transpose(0, 2, 1)).astype(np.float32)
    bd = np.exp(-slope * BLOCK).astype(np.float32)
    # correction q scale: qdecbd[h, i*BLOCK + m] = qdec[h, m] * bd[h]^i
    qdecbd = np.zeros((NH, SHARD), np.float32)
    for i in range(NCH):
        qdecbd[:, i * BLOCK:(i + 1) * BLOCK] = qdec * (bd[:, None] ** i)
    return qdec, kdec, diagT, bd, qdecbd


def _make_in_maps(hidden_states, attention_mask, slope_rate, w_qkv, w_gate,
                  w_out, norm_weight):
    BF = ml_dtypes.bfloat16
    hs = np.ascontiguousarray(np.asarray(hidden_states, np.float32)
                              .reshape(B * S, H).astype(BF))
    mask = np.ascontiguousarray(np.asarray(attention_mask, np.float32)
                                .reshape(B * S))
    w_qkv = np.ascontiguousarray(np.asarray(w_qkv, np.float32).astype(BF))
    w_gate = np.ascontiguousarray(np.asarray(w_gate, np.float32).astype(BF))
    w_out = np.ascontiguousarray(np.asarray(w_out, np.float32).astype(BF))
    nw = np.ascontiguousarray(np.asarray(norm_weight, np.float32))
    slope = np.asarray(slope_rate, np.float32)
    qdec, kdec, diagT, bd, qdecbd = _host_tables(slope)

    in_maps = []
    for c in range(N_CORES):
        r = c % GRP
        wj = np.zeros((NH, GRP), np.float32)
        for j in range(r):
            wj[:, j] = bd ** (4 * (r - 1 - j))
        in_maps.append({
            "x": np.ascontiguousarray(hs[c * SHARD:(c + 1) * SHARD]),
            "mask": np.ascontiguousarray(mask[c * SHARD:(c + 1) * SHARD]),
            "w_qkv": w_qkv, "w_gate": w_gate, "w_out": w_out, "nw": nw,
            "qdec": qdec, "kdec": kdec, "diagT": diagT, "bd": bd,
            "wj": wj, "qdecbd": qdecbd,
        })
    return in_maps


def kernel(hidden_states, attention_mask, slope_rate, w_qkv, w_gate, w_out,
           norm_weight):
    nc = _get_nc()
    in_maps = _make_in_maps(hidden_states, attention_mask, slope_rate, w_qkv,
                            w_gate, w_out, norm_weight)

    import os
    trace = bool(int(os.environ.get("KERNEL_TRACE", "0")))
    res = run_bass_kernel_spmd(nc, in_maps, core_ids=list(range(N_CORES)),
                               trace=trace)
    kernel.last_results = res
    out = np.concatenate([res.results[c]["y"] for c in range(N_CORES)], axis=0)
    return out.reshape(B, S, H)


# revision 15
# speedup vs baseline: 1.2027x; 1.0910x over previous
"""MiniMax Text01 Lightning Attention — 8-core Trainium2 Bass kernel.

Sharding: token-sharded (data parallel over B*S). Each core handles 1024
contiguous tokens of the flattened (B*S) axis = 4 blocks of 256 for one batch.
The chunk-scan dependency across shards is resolved with a per-(batch,head)
kv "contribution" AllGather within each batch's 4-core group plus a cheap
post-hoc correction term (o += (q*qdec*bd^i) @ kv_start).

Phases (per core):
  P0  PE-transpose x shard -> XT [2048k, 1024t] bf16
  P1a q/k/gate projections (bf16, lhsT=W, rhs=XT) -> silu/sigmoid -> parks
  P1b v projection (bf16, lhsT=XT, rhs=Wv) -> silu*mask -> v_sb (SBUF)
  P2  lightning attention per head (bf16 matmuls, f32 psum), local chunks
  P3  2x AllGather of local kv contributions within batch group
  P4  kv_start combine + o corrections
  P5  RMSNorm (bn_stats) + transpose o + gate multiply -> ogT
  P6  output projection -> y shard
"""

from contextlib import ExitStack

import ml_dtypes
import numpy as np

import concourse.bacc as bacc
import concourse.bass as bass
import concourse.mybir as mybir
import concourse.tile as tile
from concourse.bass_utils import run_bass_kernel_spmd
from concourse.masks import make_identity

F32 = mybir.dt.float32
BF16 = mybir.dt.bfloat16
AF = mybir.ActivationFunctionType
OP = mybir.AluOpType

B, S, H = 2, 4096, 2048
NH, HD = 16, 128
BLOCK = 256
EPS = 1e-5
N_CORES = 8
SHARD = (B * S) // N_CORES      # 1024 tokens/core
TT = SHARD // 128               # 8 token tiles of 128
NCH = SHARD // BLOCK            # 4 local chunks
KC = H // 128                   # 16 contraction chunks
GRP = N_CORES // B              # 4 cores per batch group


def _bcast_ap(src_1d: bass.AP, parts: int = 128) -> bass.AP:
    """Partition-broadcast a 1-D AP (for DMA replication)."""
    return bass.AP(tensor=src_1d.tensor, offset=src_1d.offset,
                   ap=[[0, parts]] + list(src_1d.ap))


def _rep_free(src_2d: bass.AP, times: int) -> bass.AP:
    """Insert a step-0 middle dim: [P, N] -> [P, times, N]."""
    return bass.AP(tensor=src_2d.tensor, offset=src_2d.offset,
                   ap=[src_2d.ap[0], [0, times], src_2d.ap[1]])


def _build():
    nc = bacc.Bacc("TRN2", target_bir_lowering=False, debug=False,
                   num_devices=N_CORES)

    x = nc.dram_tensor("x", [SHARD, H], BF16, kind="ExternalInput").ap()
    w_qkv = nc.dram_tensor("w_qkv", [H, 3 * H], BF16,
                           kind="ExternalInput").ap()
    w_gate = nc.dram_tensor("w_gate", [H, H], BF16,
                            kind="ExternalInput").ap()
    w_out = nc.dram_tensor("w_out", [H, H], BF16,
                           kind="ExternalInput").ap()
    nw = nc.dram_tensor("nw", [H], F32, kind="ExternalInput").ap()
    mask = nc.dram_tensor("mask", [SHARD], F32, kind="ExternalInput").ap()
    qdec = nc.dram_tensor("qdec", [NH, BLOCK], BF16,
                          kind="ExternalInput").ap()
    kdec = nc.dram_tensor("kdec", [NH, BLOCK], F32, kind="ExternalInput").ap()
    diagT = nc.dram_tensor("diagT", [NH, BLOCK, BLOCK], BF16,
                           kind="ExternalInput").ap()
    bd = nc.dram_tensor("bd", [NH], F32, kind="ExternalInput").ap()
    wj = nc.dram_tensor("wj", [NH, GRP], F32, kind="ExternalInput").ap()
    bdp = nc.dram_tensor("bdp", [NH, NCH], F32, kind="ExternalInput").ap()
    y = nc.dram_tensor("y", [SHARD, H], F32, kind="ExternalOutput").ap()

    with tile.TileContext(nc) as tc, ExitStack() as stack:
        consts = stack.enter_context(tc.tile_pool(name="consts", bufs=1))
        ident_f = consts.tile([128, 128], F32)
        make_identity(nc, ident_f)
        ident_b = consts.tile([128, 128], BF16)
        make_identity(nc, ident_b)
        nw_sb = consts.tile([128, KC], F32)
        nc.sync.dma_start(nw_sb, nw.rearrange("(a p) -> p a", p=128))
        mask_sb = consts.tile([128, TT], F32)
        nc.sync.dma_start(mask_sb, mask.rearrange("(a p) -> p a", p=128))
        bd_sb = consts.tile([128, NH], F32)
        nc.sync.dma_start(bd_sb, _bcast_ap(bd))
        wj_sb = consts.tile([128, NH, GRP], F32)
        nc.sync.dma_start(wj_sb, _bcast_ap(wj.rearrange("h j -> (h j)")
                                           ).rearrange("p (h j) -> p h j", h=NH))
        bdp_sb = consts.tile([128, NH, NCH], F32)
        nc.sync.dma_start(bdp_sb, _bcast_ap(bdp.rearrange("h i -> (h i)")
                                            ).rearrange("p (h i) -> p h i",
                                                        h=NH))
        kdec_sb = consts.tile([128, NH, 2], F32)
        nc.sync.dma_start(kdec_sb, kdec.rearrange("h (c p) -> p h c", p=128))
        eps_sb = consts.tile([128, 1], F32)
        nc.vector.memset(eps_sb, EPS)
        qd_all = consts.tile([128, NH, BLOCK], BF16)
        nc.sync.dma_start(qd_all, _bcast_ap(qdec.rearrange("h m -> (h m)")
                                            ).rearrange("p (h m) -> p h m",
                                                        h=NH))

        dram = stack.enter_context(tc.tile_pool(name="dram", bufs=1,
                                                space="DRAM"))
        qk_park = dram.tile([2, NH, 128, SHARD], BF16)   # [q/k, head, d, t]
        g_park = dram.tile([KC, 128, SHARD], BF16)       # gateT [gch, t]
        cc_in = dram.tile([NH, HD, HD], F32)
        cc_out = dram.tile([GRP, NH // 2, HD, HD], F32)
        cc_out2 = dram.tile([GRP, NH // 2, HD, HD], F32)

        v_pool = stack.enter_context(tc.tile_pool(name="vsb", bufs=1))
        v_sb = v_pool.tile([128, TT, H], BF16)           # v natural, resident

        # ---------------- P0 + P1: projections ----------------
        with (
            tc.tile_pool(name="xt", bufs=1) as xt_pool,
            tc.tile_pool(name="xin", bufs=2) as xin_pool,
            tc.tile_pool(name="wq", bufs=3) as wq_pool,
            tc.tile_pool(name="wv", bufs=2) as wv_pool,
            tc.tile_pool(name="stage", bufs=4) as stage_pool,
            tc.tile_pool(name="ps_t", bufs=2, space="PSUM") as ps_t,
            tc.tile_pool(name="ps_mm", bufs=3, space="PSUM") as ps_mm,
            tc.tile_pool(name="ps_mv", bufs=2, space="PSUM") as ps_mv,
        ):
            xt = xt_pool.tile([128, KC, SHARD], BF16)    # XT [k, t]
            for i in range(TT):
                x_in = xin_pool.tile([128, H], BF16)
                nc.sync.dma_start(x_in, x[i * 128:(i + 1) * 128, :])
                for kc in range(KC):
                    pst = ps_t.tile([128, 128], BF16, tag="t128")
                    nc.tensor.transpose(pst, x_in[:, kc * 128:(kc + 1) * 128],
                                        ident_b)
                    nc.scalar.copy(xt[:, kc, i * 128:(i + 1) * 128], pst)

            # P1a: q, k, gate (transposed outputs)
            for ch in range(3 * KC):
                if ch < 2 * KC:
                    src = w_qkv[:, ch * 128:(ch + 1) * 128]
                else:
                    c2 = ch - 2 * KC
                    src = w_gate[:, c2 * 128:(c2 + 1) * 128]
                w_t = wq_pool.tile([128, KC, 128], BF16, tag="wq")
                nc.sync.dma_start(w_t, src.rearrange("(kc kp) c -> kp kc c",
                                                     kp=128))
                for th in range(SHARD // 512):
                    psum = ps_mm.tile([128, 512], F32, tag="mm512")
                    for kc in range(KC):
                        nc.tensor.matmul(
                            psum,
                            lhsT=w_t[:, kc],
                            rhs=xt[:, kc, th * 512:(th + 1) * 512],
                            start=(kc == 0), stop=(kc == KC - 1))
                    st = stage_pool.tile([128, 512], BF16, tag="stg")
                    if ch < 2 * KC:
                        nc.scalar.activation(st, psum, AF.Silu)
                        nc.sync.dma_start(
                            qk_park[ch // KC, ch % KC][:, th * 512:(th + 1) * 512],
                            st)
                    else:
                        nc.scalar.activation(st, psum, AF.Sigmoid)
                        nc.sync.dma_start(
                            g_park[ch - 2 * KC][:, th * 512:(th + 1) * 512], st)

            # P1b: v (natural output) straight into resident SBUF
            NV = 4
            VW = H // NV
            for j in range(NV):
                wv_t = wv_pool.tile([128, KC, VW], BF16, tag="wv")
                nc.sync.dma_start(
                    wv_t,
                    w_qkv[:, 2 * H + j * VW:2 * H + (j + 1) * VW]
                    .rearrange("(kc kp) c -> kp kc c", kp=128))
                for i in range(TT):
                    psum = ps_mv.tile([128, VW], F32, tag="mm512v")
                    for kc in range(KC):
                        nc.tensor.matmul(
                            psum,
                            lhsT=xt[:, kc, i * 128:(i + 1) * 128],
                            rhs=wv_t[:, kc],
                            start=(kc == 0), stop=(kc == KC - 1))
                    vdst = v_sb[:, i, j * VW:(j + 1) * VW]
                    nc.scalar.activation(vdst, psum, AF.Silu)
                    nc.vector.tensor_scalar_mul(vdst, vdst,
                                                mask_sb[:, i:i + 1])

        # ---------------- P2-P6 ----------------
        with tc.tile_pool(name="ogt", bufs=1) as ogt_pool:
            ogT = ogt_pool.tile([128, KC, SHARD], BF16)
            with tc.tile_pool(name="osb", bufs=1) as o_pool:
                o_sb = o_pool.tile([128, TT, H], F32)
                with (
                    tc.tile_pool(name="att", bufs=3) as att,
                    tc.tile_pool(name="attw", bufs=3) as attw,
                    tc.tile_pool(name="kvp", bufs=3) as kvp,
                    tc.tile_pool(name="ps_at", bufs=2, space="PSUM") as ps_at,
                    tc.tile_pool(name="ps_o", bufs=2, space="PSUM") as ps_o,
                    tc.tile_pool(name="ps_kv", bufs=2, space="PSUM") as ps_kv,
                    tc.tile_pool(name="ps_sm", bufs=2, space="PSUM") as ps_sm,
                ):
                    # ---- P2: local attention ----
                    for h in range(NH):
                        qT = att.tile([128, SHARD], BF16, tag="qh")
                        nc.sync.dma_start(qT, qk_park[0, h])
                        kT = att.tile([128, SHARD], BF16, tag="kh")
                        nc.sync.dma_start(kT, qk_park[1, h])
                        dgT = att.tile([128, 2, BLOCK], BF16, tag="dg")
                        nc.sync.dma_start(dgT,
                                          diagT[h].rearrange("(c p) m -> p c m",
                                                             p=128))

                        kv_sb = kvp.tile([128, HD], F32, tag="kv")
                        kv_bf = kvp.tile([128, HD], BF16, tag="kvb")

                        for i in range(NCH):
                            if i > 0:
                                qTd = attw.tile([128, BLOCK], BF16, tag="qtd")
                                nc.vector.tensor_tensor(
                                    qTd, qT[:, i * BLOCK:(i + 1) * BLOCK],
                                    qd_all[:, h], OP.mult)
                            at_sb = attw.tile([128, 2, BLOCK], BF16,
                                              tag="atsb")
                            for p in range(2):
                                ps = ps_at.tile([128, BLOCK], F32, tag="at")
                                nc.tensor.matmul(
                                    ps,
                                    lhsT=kT[:, i * BLOCK + p * 128:
                                            i * BLOCK + (p + 1) * 128],
                                    rhs=qT[:, i * BLOCK:(i + 1) * BLOCK],
                                    start=True, stop=True)
                                nc.vector.tensor_tensor(at_sb[:, p], ps,
                                                        dgT[:, p], OP.mult)
                            kd = attw.tile([128, 2, HD], BF16, tag="kd")
                            for p in range(2):
                                pst = ps_sm.tile([128, 128], BF16, tag="sm")
                                nc.tensor.transpose(
                                    pst, kT[:, i * BLOCK + p * 128:
                                            i * BLOCK + (p + 1) * 128],
                                    ident_b)
                                nc.scalar.activation(
                                    kd[:, p], pst, AF.Copy,
                                    scale=kdec_sb[:, h, p:p + 1])
                            for mh in range(2):
                                pso = ps_o.tile([128, HD], F32, tag="o")
                                if i > 0:
                                    nc.tensor.matmul(
                                        pso,
                                        lhsT=qTd[:, mh * 128:(mh + 1) * 128],
                                        rhs=kv_bf, start=True, stop=False)
                                for p in range(2):
                                    nc.tensor.matmul(
                                        pso,
                                        lhsT=at_sb[:, p,
                                                   mh * 128:(mh + 1) * 128],
                                        rhs=v_sb[:, 2 * i + p,
                                                 h * HD:(h + 1) * HD],
                                        start=(i == 0 and p == 0),
                                        stop=(p == 1))
                                nc.scalar.copy(
                                    o_sb[:, 2 * i + mh, h * HD:(h + 1) * HD],
                                    pso)
                            pskv = ps_kv.tile([128, HD], F32, tag="kvps")
                            for p in range(2):
                                nc.tensor.matmul(
                                    pskv, lhsT=kd[:, p],
                                    rhs=v_sb[:, 2 * i + p,
                                             h * HD:(h + 1) * HD],
                                    start=(p == 0), stop=(p == 1))
                            if i == 0:
                                nc.vector.tensor_copy(kv_sb, pskv)
                            else:
                                nc.vector.scalar_tensor_tensor(
                                    kv_sb, in0=kv_sb, scalar=bd_sb[:, h:h + 1],
                                    in1=pskv, op0=OP.mult, op1=OP.add)
                            if i < NCH - 1:
                                nc.scalar.copy(kv_bf, kv_sb)
                        nc.sync.dma_start(cc_in[h], kv_sb)
                        if h == NH // 2 - 1:
                            nc.gpsimd.collective_compute(
                                "AllGather", OP.bypass,
                                replica_groups=[[0, 1, 2, 3], [4, 5, 6, 7]],
                                ins=[cc_in[0:NH // 2].opt()],
                                outs=[cc_out.opt()])
                        elif h == NH - 1:
                            nc.gpsimd.collective_compute(
                                "AllGather", OP.bypass,
                                replica_groups=[[0, 1, 2, 3], [4, 5, 6, 7]],
                                ins=[cc_in[NH // 2:].opt()],
                                outs=[cc_out2.opt()])

                    # ---- P4: kv_start combine (batched) + o corrections ----
                    NHH = NH // 2
                    kvs_all = {}
                    for half, cco in ((0, cc_out), (1, cc_out2)):
                        kvs = kvp.tile([128, NHH, HD], F32, tag="kvsall")
                        nc.vector.memset(kvs, 0.0)
                        for j in range(GRP):
                            cj = attw.tile([128, NHH, HD], F32, tag="ccj")
                            nc.sync.dma_start(
                                cj, cco[j].rearrange("h p e -> p h e"))
                            tmp = attw.tile([128, NHH, HD], F32, tag="cct")
                            wjs = wj_sb[:, half * NHH:(half + 1) * NHH,
                                        j:j + 1]
                            wj_b = bass.AP(
                                tensor=wjs.tensor, offset=wjs.offset,
                                ap=[wjs.ap[0], wjs.ap[1], [0, HD]])
                            nc.vector.tensor_tensor(tmp, cj, wj_b, OP.mult)
                            nc.vector.tensor_tensor(kvs, kvs, tmp, OP.add)
                        kvs_all[half] = kvs

                    for h in range(NH):
                        kvs_h = kvs_all[h // NHH][:, h % NHH, :]
                        qT2 = att.tile([128, SHARD], BF16, tag="qh")
                        nc.sync.dma_start(qT2, qk_park[0, h])
                        qTdc = att.tile([128, NCH, BLOCK], BF16, tag="qtdc")
                        nc.vector.tensor_tensor(
                            qTdc, qT2.rearrange("p (c m) -> p c m", m=BLOCK),
                            _rep_free(qd_all[:, h], NCH), OP.mult)
                        kvs_bf = kvp.tile([128, NCH, HD], BF16, tag="kvsb")
                        for i in range(NCH):
                            nc.vector.tensor_scalar_mul(
                                kvs_bf[:, i], kvs_h, bdp_sb[:, h, i:i + 1])
                        for m in range(TT):
                            ps = ps_sm.tile([128, 128], F32, tag="sm")
                            nc.tensor.matmul(
                                ps, lhsT=qTdc[:, m // 2,
                                              (m % 2) * 128:(m % 2 + 1) * 128],
                                rhs=kvs_bf[:, m // 2], start=True, stop=True)
                            osl = o_sb[:, m, h * HD:(h + 1) * HD]
                            nc.vector.tensor_tensor(osl, osl, ps, OP.add)

                # ---- P5: norm, then transpose+gate -> ogT; then P6 ----
                with (
                    tc.tile_pool(name="nrm", bufs=4) as nrm,
                    tc.tile_pool(name="gld", bufs=3) as gld,
                    tc.tile_pool(name="ps5", bufs=3, space="PSUM") as ps5,
                    tc.tile_pool(name="wo", bufs=2) as wo_pool,
                    tc.tile_pool(name="ost", bufs=4) as ost_pool,
                    tc.tile_pool(name="ps_mo", bufs=4, space="PSUM") as ps_mo,
                ):
                    for i in range(TT):
                        stats = nrm.tile([128, 4, 6], F32, tag="bst")
                        for sg in range(4):
                            nc.vector.bn_stats(
                                stats[:, sg],
                                o_sb[:, i, sg * 512:(sg + 1) * 512])
                        mv = nrm.tile([128, 2], F32, tag="mv")
                        nc.vector.bn_aggr(mv, stats)
                        msq = nrm.tile([128, 1], F32, tag="msq")
                        nc.vector.tensor_tensor(msq, mv[:, 0:1], mv[:, 0:1],
                                                OP.mult)
                        nc.vector.tensor_tensor(msq, msq, mv[:, 1:2], OP.add)
                        std = nrm.tile([128, 1], F32, tag="std")
                        nc.scalar.activation(std, msq, AF.Sqrt,
                                             bias=eps_sb[:, 0:1])
                        rstd = nrm.tile([128, 1], F32, tag="rstd")
                        nc.vector.reciprocal(rstd, std)
                        nc.vector.tensor_scalar_mul(o_sb[:, i, :],
                                                    o_sb[:, i, :], rstd)
                    for kc in range(KC):
                        g_kc = gld.tile([128, SHARD], BF16, tag="gkc")
                        nc.sync.dma_start(g_kc, g_park[kc])
                        for i in range(TT):
                            pst = ps5.tile([128, 128], F32, tag="tog")
                            nc.tensor.transpose(
                                pst, o_sb[:, i, kc * 128:(kc + 1) * 128],
                                ident_f)
                            nc.vector.scalar_tensor_tensor(
                                ogT[:, kc, i * 128:(i + 1) * 128],
                                in0=pst, scalar=nw_sb[:, kc:kc + 1],
                                in1=g_kc[:, i * 128:(i + 1) * 128],
                                op0=OP.mult, op1=OP.mult)

                    # ---- P6: out projection ----
                    for j in range(4):
                        wo_t = wo_pool.tile([128, KC, 512], BF16, tag="wo")
                        nc.sync.dma_start(
                            wo_t, w_out[:, j * 512:(j + 1) * 512]
                            .rearrange("(kc kp) c -> kp kc c", kp=128))
                        for i in range(TT):
                            psum = ps_mo.tile([128, 512], F32, tag="mo")
                            for kc in range(KC):
                                nc.tensor.matmul(
                                    psum,
                                    lhsT=ogT[:, kc, i * 128:(i + 1) * 128],
                                    rhs=wo_t[:, kc],
                                    start=(kc == 0), stop=(kc == KC - 1))
                            ost = ost_pool.tile([128, 512], F32, tag="ost")
                            nc.scalar.copy(ost, psum)
                            nc.sync.dma_start(
                                y[i * 128:(i + 1) * 128,
                                  j * 512:(j + 1) * 512], ost)

    nc.compile()
    return nc


_CACHED = None


def _get_nc():
    global _CACHED
    if _CACHED is None:
        _CACHED = _build()
    return _CACHED


def _host_tables(slope: np.ndarray):
    slope = slope.astype(np.float32)
    ar = np.arange(BLOCK, dtype=np.float32) + 1.0
    qdec = np.exp(-slope[:, None] * ar[None, :]).astype(np.float32)
    kdec = np.exp(-slope[:, None] * (BLOCK - ar)[None, :]).astype(np.float32)
    idx = ar[:, None] - ar[None, :]
    m2 = (idx >= 0).astype(np.float32)
    diag = np.exp(-slope[:, None, None] * (idx * m2)[None]) * m2[None]
    diagT = np.ascontiguousarray(diag.transpose(0, 2, 1)).astype(np.float32)
    bd = np.exp(-slope * BLOCK).astype(np.float32)
    bdp = np.stack([bd ** i for i in range(NCH)], axis=1).astype(np.float32)
    return qdec, kdec, diagT, bd, bdp


def _make_in_maps(hidden_states, attention_mask, slope_rate, w_qkv, w_gate,
                  w_out, norm_weight):
    BF = ml_dtypes.bfloat16
    hs = np.ascontiguousarray(np.asarray(hidden_states, np.float32)
                              .reshape(B * S, H).astype(BF))
    mask = np.ascontiguousarray(np.asarray(attention_mask, np.float32)
                                .reshape(B * S))
    w_qkv = np.ascontiguousarray(np.asarray(w_qkv, np.float32).astype(BF))
    w_gate = np.ascontiguousarray(np.asarray(w_gate, np.float32).astype(BF))
    w_out = np.ascontiguousarray(np.asarray(w_out, np.float32).astype(BF))
    nw = np.ascontiguousarray(np.asarray(norm_weight, np.float32))
    slope = np.asarray(slope_rate, np.float32)
    qdec, kdec, diagT, bd, bdp = _host_tables(slope)
    qdec_bf = np.ascontiguousarray(qdec.astype(BF))
    diagT_bf = np.ascontiguousarray(diagT.astype(BF))

    in_maps = []
    for c in range(N_CORES):
        r = c % GRP
        wj = np.zeros((NH, GRP), np.float32)
        for j in range(r):
            wj[:, j] = bd ** (4 * (r - 1 - j))
        in_maps.append({
            "x": np.ascontiguousarray(hs[c * SHARD:(c + 1) * SHARD]),
            "mask": np.ascontiguousarray(mask[c * SHARD:(c + 1) * SHARD]),
            "w_qkv": w_qkv, "w_gate": w_gate, "w_out": w_out, "nw": nw,
            "qdec": qdec_bf, "kdec": kdec, "diagT": diagT_bf, "bd": bd,
            "wj": wj, "bdp": bdp,
        })
    return in_maps


def kernel(hidden_states, attention_mask, slope_rate, w_qkv, w_gate, w_out,
           norm_weight):
    nc = _get_nc()
    in_maps = _make_in_maps(hidden_states, attention_mask, slope_rate, w_qkv,
                            w_gate, w_out, norm_weight)

    import os
    trace = bool(int(os.environ.get("KERNEL_TRACE", "0")))
    res = run_bass_kernel_spmd(nc, in_maps, core_ids=list(range(N_CORES)),
                               trace=trace)
    kernel.last_results = res
    out = np.concatenate([res.results[c]["y"] for c in range(N_CORES)], axis=0)
    return out.reshape(B, S, H)
